# revision 1
# baseline (speedup 1.0000x reference)
"""CrossVariableAttention Bass/Tile kernel for TRN2.

Per-core program (data parallel over batch, one batch element per core).
Two host-side algebraic fusions cut the on-chip matmul count from 832 to 720:

  scores:  S = (q+bq')(k+bk)^T.  Terms constant along the softmax axis (m)
           cancel, so with B := Wq'.Wk^T (host) and u := Wk.bq' (host):
             S^T[m,n] ~ X[:,m]^T.B^T.X[:,n] + r[m],  r = u^T.X
           C := B^T.X is one 512x512x2048 matmul; r folds into the exp bias.
  output:  (P.V).Wp = P.(V.Wp), and V.Wp = X^T.(Wv.Wp) + bv.Wp, so with
           Wvp := Wv.Wp and bvp := bv.Wp (host):
             VP := X^T.Wvp + bvp;  y_un^T = VP^T.Pt;  y = y_un*recip + bp
           The bvp term is exact after softmax normalization (sum_m P = 1).

Per-core dataflow (all matmuls in float32r = full-rate fp32, ~1e-4 rel err):
  phase 1:  C [d,n] = WB^T.X;  r [1,n] = u^T.X;  VP [m,d] = X^T.Wvp + bvp
  phase 2, per slab of 512 queries:
    S^T = X^T.C        [m, 512]   (16 psum tiles)
    Pt  = exp(S^T + r[m])         (ACT, bias per partition)
    den = ones^T.Pt    [1, 512]   (16 M=1 matmuls, accumulated)
    recip_bc = 1/den broadcast to [128, 512]  (DRAM bounce)
    Y^T = VP^T.Pt      [dout, 512]
    y   = Y^T*recip_bc + bp  -> DMA out ([L, N] layout, no transposes anywhere)
"""

from contextlib import ExitStack

import concourse.bass as bass
import concourse.mybir as mybir
import concourse.tile as tile
from concourse.bass import ds
from concourse.vector_clock import ScopedClock

F32 = mybir.dt.float32
F32R = mybir.dt.float32r
AF = mybir.ActivationFunctionType

P = 128
D = 512
N = 2048
DCH = D // P         # 4 d chunks
NCH = N // P         # 16 token chunks (m)
NSLAB = N // 512     # 4 query slabs


# ---------------------------------------------------------------------------
# The walrus build in this env accepts at most ONE sync wait per instruction
# (setupSyncWait: "Too many sync wait commands").  Tile attaches several.
# Fix: split excess waits onto engine-local NOPs placed just before the
# instruction (same engine => same stream order => identical semantics).
MAX_WAITS_PER_INST = 1


class SplitDrainTileContext(tile.TileContext):
    def _drain_and_barrier(self, tick_clock, wait_clock):
        nc = self.nc
        probe = nc.sync.nop(nofuse=True, hint="split_drain_waits")
        wait_clock.add_sem_waits(
            probe.ins, ScopedClock({None: tick_clock.global_clock})
        )
        waits = list(probe.ins.sync_info.on_wait)
        probe.ins.sync_info.on_wait = waits[:MAX_WAITS_PER_INST]
        for i in range(MAX_WAITS_PER_INST, len(waits), MAX_WAITS_PER_INST):
            extra = nc.sync.nop(nofuse=True, hint="split_drain_waits")
            extra.ins.sync_info = mybir.SyncInfo(
                on_wait=waits[i : i + MAX_WAITS_PER_INST], on_update=[]
            )
        nc.sync.drain()
        nc.all_engine_barrier()
        assert self.sems is not None
        popped = nc._tile_sem_poison_stack.pop()
        assert popped is self._sem_poison
        nc.clear_and_free_semaphores(list(self.sems.allocated().values()))
        nc.all_engine_barrier()


def split_sync_waits(nc, max_waits=MAX_WAITS_PER_INST):
    n_split = 0
    for fn in nc.m.functions:
        for bb in fn.blocks:
            insts = list(bb.instructions)
            out = []
            changed = False
            for inst in insts:
                si = getattr(inst, "sync_info", None)
                if si is not None:
                    waits = list(si.on_wait or [])
                    if len(waits) > max_waits:
                        changed = True
                        for j, w in enumerate(waits[: len(waits) - max_waits]):
                            out.append(
                                mybir.InstNoOp(
                                    name=f"{inst.name}-sw{j}",
                                    engine=inst.engine,
                                    bass_nofuse=True,
                                    sync_info=mybir.SyncInfo(
                                        on_wait=[w], on_update=[]
                                    ),
                                )
                            )
                            n_split += 1
                        si.on_wait = waits[len(waits) - max_waits :]
                out.append(inst)
            if changed:
                bb.instructions = out
    return n_split


def build_nc():
    nc = bass.Bass()

    x = nc.declare_dram_parameter("x", [D, N], F32R, isOutput=False)
    wb = nc.declare_dram_parameter("wb", [DCH, P, DCH, P], F32R, isOutput=False)
    wvp = nc.declare_dram_parameter("wvp", [D, D], F32R, isOutput=False)
    u = nc.declare_dram_parameter("u", [D], F32R, isOutput=False)
    ones_in = nc.declare_dram_parameter("ones", [P, 1], F32R, isOutput=False)
    bvp = nc.declare_dram_parameter("bvp", [D], F32, isOutput=False)
    bp = nc.declare_dram_parameter("bp", [D], F32, isOutput=False)
    y = nc.declare_dram_parameter("y", [D, N], F32, isOutput=True)
    r_dram = nc.dram_tensor("r_scratch", [N], F32)
    recip_dram = nc.dram_tensor("recip_scratch", [NSLAB, 512], F32)

    with SplitDrainTileContext(nc) as tc, ExitStack() as ctx:
        consts = ctx.enter_context(tc.tile_pool(name="consts", bufs=1))
        big = ctx.enter_context(tc.tile_pool(name="big", bufs=1))
        small = ctx.enter_context(tc.tile_pool(name="small", bufs=3))

        bp_sb = consts.tile([P, DCH], F32, tag="bp")
        u_sb = consts.tile([P, DCH], F32R, tag="u")
        ones = consts.tile([P, 1], F32R, tag="ones")
        bvp_bc = consts.tile([P, D], F32, tag="bvp")
        wvp_sb = consts.tile([P, DCH, D], F32R, tag="wvp")
        rcol_sb = consts.tile([P, NCH], F32, tag="rcol")

        # --- persistent big tensors --------------------------------------
        c_sb = big.tile([P, DCH, N], F32R, tag="c")
        vp_sb = big.tile([P, NCH, D], F32R, tag="vp")
        x_tiles = []
        for nb in range(NSLAB):
            xt_nb = big.tile([P, DCH, 512], F32R, tag=f"x{nb}")
            x_tiles.append(xt_nb)

        # --- phase 1: C, r, VP (input DMAs just-in-time) ------------------
        with tc.tile_pool(name="xin", bufs=1) as xin, \
             tc.tile_pool(name="ps1", bufs=4, space="PSUM") as ps1, \
             tc.tile_pool(name="ps_r", bufs=2, space="PSUM") as ps_r:
            wb_tiles = []
            for oc in range(DCH):
                wbt = xin.tile([P, DCH, P], F32R, tag=f"wb{oc}")
                wb_tiles.append(wbt)
            nc.sync.dma_start(out=wb_tiles[0], in_=wb[0])
            nc.sync.dma_start(out=u_sb, in_=u.rearrange("(c p) -> p c", p=P))
            x_re = x.rearrange("(c p) n -> p c n", p=P)
            nc.sync.dma_start(out=x_tiles[0], in_=x_re[:, :, ds(0, 512)])
            for oc in range(1, DCH):
                nc.sync.dma_start(out=wb_tiles[oc], in_=wb[oc])
            for nb in range(1, NSLAB):
                nc.sync.dma_start(
                    out=x_tiles[nb], in_=x_re[:, :, ds(nb * 512, 512)]
                )
            nc.sync.dma_start(out=wvp_sb, in_=wvp.rearrange("(c p) o -> p c o", p=P))
            nc.sync.dma_start(out=bp_sb, in_=bp.rearrange("(c p) -> p c", p=P))
            nc.sync.dma_start(out=ones, in_=ones_in[:, :])
            bvp_ap = bvp[:]
            nc.sync.dma_start(
                out=bvp_bc,
                in_=bass.AP(
                    tensor=bvp_ap.tensor, offset=bvp_ap.offset,
                    ap=[[0, P], bvp_ap.ap[0]],
                ),
            )

            for nb in range(NSLAB):
                # C[:, :, slab] = WB^T . X[:, slab]
                for oc in range(DCH):
                    ps = ps1.tile([P, 512], F32, tag="ps1")
                    for ic in range(DCH):
                        nc.tensor.matmul(
                            ps,
                            wb_tiles[oc][:, ic, :],
                            x_tiles[nb][:, ic, :],
                            start=(ic == 0),
                            stop=(ic == DCH - 1),
                        )
                    nc.scalar.copy(out=c_sb[:, oc, ds(nb * 512, 512)], in_=ps)
                # r[slab] = u^T . X[:, slab]
                psr = ps_r.tile([1, 512], F32, tag="psr")
                for ic in range(DCH):
                    nc.tensor.matmul(
                        psr,
                        u_sb[:, ic : ic + 1],
                        x_tiles[nb][:, ic, :],
                        start=(ic == 0),
                        stop=(ic == DCH - 1),
                    )
                r_sb = small.tile([1, 512], F32, tag="rsb")
                nc.vector.tensor_copy(out=r_sb, in_=psr)
                nc.sync.dma_start(out=r_dram[ds(nb * 512, 512)], in_=r_sb)

            # VP = X^T . Wvp + bvp
            for mc in range(NCH):
                ps = ps1.tile([P, 512], F32, tag="ps1")
                for ic in range(DCH):
                    nc.tensor.matmul(
                        ps,
                        x_tiles[mc // 4][:, ic, ds((mc % 4) * P, P)],
                        wvp_sb[:, ic, :],
                        start=(ic == 0),
                        stop=(ic == DCH - 1),
                    )
                nc.vector.tensor_add(out=vp_sb[:, mc, :], in0=ps, in1=bvp_bc)

            # r in column layout [128, 16]: rcol[p, mc] = r[mc*128 + p]
            nc.sync.dma_start(
                out=rcol_sb, in_=r_dram.rearrange("(c p) -> p c", p=P)
            )

        # --- phase 2: attention, per slab of 512 queries ------------------
        with tc.tile_pool(name="pt", bufs=20) as pt_pool, \
             tc.tile_pool(name="outp", bufs=4) as outp, \
             tc.tile_pool(name="ps_st", bufs=4, space="PSUM") as ps_st, \
             tc.tile_pool(name="ps_den", bufs=1, space="PSUM") as ps_den, \
             tc.tile_pool(name="ps_y", bufs=3, space="PSUM") as ps_y:
            for nb in range(NSLAB):
                nsl = ds(nb * 512, 512)

                # S^T tiles + exp(S + r); DVE accumulates Pt pairs so the
                # denominator needs only ONE M=1 matmul instead of 16
                pt_tiles = []
                acc = small.tile([P, 512], F32R, tag="denacc")
                for mc in range(NCH):
                    ps = ps_st.tile([P, 512], F32, tag="st")
                    for ic in range(DCH):
                        nc.tensor.matmul(
                            ps,
                            x_tiles[mc // 4][:, ic, ds((mc % 4) * P, P)],
                            c_sb[:, ic, nsl],
                            start=(ic == 0),
                            stop=(ic == DCH - 1),
                        )
                    pt = pt_pool.tile([P, 512], F32R, tag="pt")
                    nc.scalar.activation(
                        out=pt,
                        in_=ps,
                        func=AF.Exp,
                        bias=rcol_sb[:, mc : mc + 1],
                        scale=1.0,
                    )
                    pt_tiles.append(pt)
                    if mc == 1:
                        nc.vector.tensor_add(
                            out=acc,
                            in0=pt_tiles[0].bitcast(F32),
                            in1=pt_tiles[1].bitcast(F32),
                        )
                    elif mc > 1:
                        nc.vector.tensor_add(
                            out=acc,
                            in0=acc.bitcast(F32),
                            in1=pt.bitcast(F32),
                        )

                # denominator: single M=1 ones-matmul over the DVE partial sum
                ps_d = ps_den.tile([1, 512], F32, tag="den")
                nc.tensor.matmul(
                    ps_d, ones[:, :], acc[:, :], start=True, stop=True
                )
                den_sb = small.tile([1, 512], F32, tag="densb")
                nc.vector.tensor_copy(out=den_sb, in_=ps_d)
                nc.sync.dma_start(out=recip_dram[nb], in_=den_sb)
                recip_bc = small.tile([P, 512], F32, tag="recip_bc")
                rd = recip_dram[nb]
                nc.sync.dma_start(
                    out=recip_bc,
                    in_=bass.AP(
                        tensor=rd.tensor, offset=rd.offset,
                        ap=[[0, P], rd.ap[-1]],
                    ),
                )
                nc.vector.reciprocal(out=recip_bc, in_=recip_bc)

                # Y^T = VP^T . Pt, then normalize + bias + store
                for oc in range(DCH):
                    ps = ps_y.tile([P, 512], F32, tag="y")
                    for mc in range(NCH):
                        nc.tensor.matmul(
                            ps,
                            vp_sb[:, mc, ds(oc * P, P)],
                            pt_tiles[mc][:, :],
                            start=(mc == 0),
                            stop=(mc == NCH - 1),
                        )
                    t = outp.tile([P, 512], F32, tag="out")
                    nc.vector.tensor_tensor(
                        out=t, in0=ps, in1=recip_bc, op=mybir.AluOpType.mult
                    )
                    nc.vector.tensor_scalar_add(
                        out=t, in0=t, scalar1=bp_sb[:, oc : oc + 1]
                    )
                    nc.sync.dma_start(out=y[ds(oc * P, P), nsl], in_=t)

    split_sync_waits(nc)
    return nc


import numpy as np
from concourse.bass_utils import run_bass_kernel_spmd

B = 8

_NC_CACHE = None


def _get_nc():
    global _NC_CACHE
    if _NC_CACHE is None:
        _NC_CACHE = build_nc()
    return _NC_CACHE


def _make_in_maps(inputs):
    x = np.asarray(inputs["x"], np.float32)
    W_qkv = np.asarray(inputs["W_qkv"], np.float64)
    b_qkv = np.asarray(inputs["b_qkv"], np.float64)
    W_proj = np.asarray(inputs["W_proj"], np.float64)
    b_proj = np.asarray(inputs["b_proj"], np.float64)

    s = 1.0 / np.sqrt(np.float64(D))
    wq_s = W_qkv[:, :D] * s
    bq_s = b_qkv[:D] * s
    wk = W_qkv[:, D : 2 * D]
    wv = W_qkv[:, 2 * D :]
    bv = b_qkv[2 * D :]

    shared = {
        "wb": np.ascontiguousarray(
            (wq_s @ wk.T).astype(np.float32)
            .reshape(4, 128, 4, 128).transpose(2, 1, 0, 3)
        ),
        "wvp": np.ascontiguousarray((wv @ W_proj).astype(np.float32)),
        "u": np.ascontiguousarray((wk @ bq_s).astype(np.float32)),
        "bvp": np.ascontiguousarray((bv @ W_proj).astype(np.float32)),
        "bp": np.ascontiguousarray(b_proj.astype(np.float32)),
        "ones": np.ones((P, 1), np.float32),
    }
    return [{"x": np.ascontiguousarray(x[b]), **shared} for b in range(B)]


def kernel(**inputs):
    nc = _get_nc()
    in_maps = _make_in_maps(inputs)
    res = run_bass_kernel_spmd(nc, in_maps, core_ids=list(range(B)))
    return np.stack([res.results[b]["y"] for b in range(B)]).astype(np.float32)



# revision 9
# speedup vs baseline: 1.3633x; 1.3633x over previous
"""CrossVariableAttention Bass/Tile kernel for TRN2 (fp8 DoubleRow v2).

Per-core program (data parallel over batch, one batch element per core).

Host-side algebra (input-independent weight transforms only):
  B   := Wq'.Wk^T, u := Wk.bq'   (so S^T[m,n] = X[:,m]^T B^T X[:,n] + r[m],
                                   r = u^T X computed on device)
  Wvp := Wv.Wproj                 (V.Wproj = X^T.Wvp + bvp)
  bp* := bproj + bv.Wproj         (bvp is exact after softmax normalization)

r-folding: exp(S+r) = exp(S).w with w := exp(r) applied to the VP rows and
to the denominator weights. This removes the per-partition bias from the
exp activation, letting one ACT instruction process psum PAIRS [128,2,512]
(adjacent mc parities) and emit the exact [p, 2, n] layout the fp8
DoubleRow Y-matmul consumes.

fp8 path (e4m3, all heavy matmuls in DoubleRow perf mode, 2x PE rate):
  S^T = q8(X)^T.q8(64 C)/64   C from fp32r matmul, quantized on drain
  Pt  = q8(exp(S^T))          ACT exp with scale=1/64, fp8 output
  den = q8(w)^T.Pt            DoubleRow ones-style matmul (M=2 padded)
  Y^T = q8(w.VP)^T.Pt + bp* (x) den   rank-1 fp32r bias matmul into psum
  y   = Y^T * bcast(1/den)    DVE, then DMA out

Phases (PE / ACT overlap; ACT exp = 32 pair instructions is the
phase-2a critical path, so VP + den matmuls interleave into PE idle):
  P1 : r (+exp->w chain), C (fp32r, stationary-reuse), drains -> cq fp8
  P2a: nbh-major S DoubleRow matmuls + VP fp32r matmuls + den DR matmuls,
       ACT exp pairs -> pt fp8 (8 pair tiles [128,2,2048])
  P2b: Y DoubleRow oc-major (4 psum banks) + rank-1 bias matmul,
       recip chain (dram bounce, [128,16] reciprocal), DVE normalize, DMA
"""

from contextlib import ExitStack

import concourse.bass as bass
import concourse.mybir as mybir
import concourse.tile as tile
from concourse.bass import ds
from concourse.vector_clock import ScopedClock

F32 = mybir.dt.float32
F32R = mybir.dt.float32r
F8 = mybir.dt.float8e4
AF = mybir.ActivationFunctionType
DR = mybir.MatmulPerfMode.DoubleRow

P = 128
D = 512
N = 2048
DCH = D // P         # 4 d chunks
NCH = N // P         # 16 token chunks (m)
NSLAB = N // 512     # 4 slabs
CSCALE = 64.0        # fp8 scale for C


# ---------------------------------------------------------------------------
# The walrus build in this env accepts at most ONE sync wait per instruction
# (setupSyncWait: "Too many sync wait commands").  Tile attaches several.
# Fix: split excess waits onto engine-local NOPs placed just before the
# instruction (same engine => same stream order => identical semantics).
MAX_WAITS_PER_INST = 1


class SplitDrainTileContext(tile.TileContext):
    def _drain_and_barrier(self, tick_clock, wait_clock):
        nc = self.nc
        probe = nc.sync.nop(nofuse=True, hint="split_drain_waits")
        wait_clock.add_sem_waits(
            probe.ins, ScopedClock({None: tick_clock.global_clock})
        )
        waits = list(probe.ins.sync_info.on_wait)
        probe.ins.sync_info.on_wait = waits[:MAX_WAITS_PER_INST]
        for i in range(MAX_WAITS_PER_INST, len(waits), MAX_WAITS_PER_INST):
            extra = nc.sync.nop(nofuse=True, hint="split_drain_waits")
            extra.ins.sync_info = mybir.SyncInfo(
                on_wait=waits[i : i + MAX_WAITS_PER_INST], on_update=[]
            )
        nc.sync.drain()
        nc.all_engine_barrier()
        assert self.sems is not None
        popped = nc._tile_sem_poison_stack.pop()
        assert popped is self._sem_poison
        nc.clear_and_free_semaphores(list(self.sems.allocated().values()))
        nc.all_engine_barrier()


def split_sync_waits(nc, max_waits=MAX_WAITS_PER_INST):
    n_split = 0
    for fn in nc.m.functions:
        for bb in fn.blocks:
            insts = list(bb.instructions)
            out = []
            changed = False
            for inst in insts:
                si = getattr(inst, "sync_info", None)
                if si is not None:
                    waits = list(si.on_wait or [])
                    if len(waits) > max_waits:
                        changed = True
                        for j, w in enumerate(waits[: len(waits) - max_waits]):
                            out.append(
                                mybir.InstNoOp(
                                    name=f"{inst.name}-sw{j}",
                                    engine=inst.engine,
                                    bass_nofuse=True,
                                    sync_info=mybir.SyncInfo(
                                        on_wait=[w], on_update=[]
                                    ),
                                )
                            )
                            n_split += 1
                        si.on_wait = waits[len(waits) - max_waits :]
                out.append(inst)
            if changed:
                bb.instructions = out
    return n_split


def build_nc():
    nc = bass.Bass()

    x = nc.declare_dram_parameter("x", [D, N], F32R, isOutput=False)
    xq = nc.declare_dram_parameter("xq", [P, 2, 2, N], F8, isOutput=False)
    wb = nc.declare_dram_parameter("wb", [DCH, P, DCH, P], F32R, isOutput=False)
    wvp = nc.declare_dram_parameter("wvp", [D, D], F32R, isOutput=False)
    u = nc.declare_dram_parameter("u", [D], F32R, isOutput=False)
    bptot = nc.declare_dram_parameter("bptot", [1, D], F32R, isOutput=False)
    y = nc.declare_dram_parameter("y", [D, N], F32, isOutput=True)
    w_dram = nc.dram_tensor("w_scratch", [N], F32)
    den_dram = nc.dram_tensor("den_scratch", [N], F32R)
    recip_dram = nc.dram_tensor("recip_scratch", [N], F32)

    with SplitDrainTileContext(nc) as tc, ExitStack() as ctx:
        consts = ctx.enter_context(tc.tile_pool(name="consts", bufs=1))
        big = ctx.enter_context(tc.tile_pool(name="big", bufs=1))
        small = ctx.enter_context(tc.tile_pool(name="small", bufs=4))

        u_sb = consts.tile([P, DCH], F32R, tag="u")
        wvp_sb = consts.tile([P, DCH, D], F32R, tag="wvp")
        bptot_sb = consts.tile([1, D], F32R, tag="bptot")
        wcol = consts.tile([P, NCH], F32, tag="wcol")
        wq8 = consts.tile([P, NCH, 16], F8, tag="wq8")
        den_row = consts.tile([1, N], F32R, tag="denrow")

        # --- persistent big tensors --------------------------------------
        x_tiles = [big.tile([P, DCH, 512], F32R, tag=f"x{nb}", name=f"x{nb}") for nb in range(NSLAB)]
        xq_sb = big.tile([P, 2, 2, N], F8, tag="xq")
        cq_sb = big.tile([P, 2, 2, N], F8, tag="cq")
        vpq_sb = big.tile([P, NCH, D], F8, tag="vpq")
        pt_tiles = [big.tile([P, 2, N], F8, tag=f"pt{k}", name=f"pt{k}") for k in range(NCH // 2)]
        recip_bc = [big.tile([P, 512], F32, tag=f"rbc{nb}", name=f"rbc{nb}") for nb in range(NSLAB)]

        # --- input DMAs (order = priority) -------------------------------
        wb_tiles = [consts.tile([P, DCH, P], F32R, tag=f"wb{oc}", name=f"wb{oc}") for oc in range(DCH)]
        x_re = x.rearrange("(c p) n -> p c n", p=P)
        nc.sync.dma_start(out=wb_tiles[0], in_=wb[0])
        nc.sync.dma_start(out=u_sb, in_=u.rearrange("(c p) -> p c", p=P))
        nc.sync.dma_start(out=x_tiles[0], in_=x_re[:, :, ds(0, 512)])
        for oc in range(1, DCH):
            nc.sync.dma_start(out=wb_tiles[oc], in_=wb[oc])
        for nb in range(1, NSLAB):
            nc.sync.dma_start(out=x_tiles[nb], in_=x_re[:, :, ds(nb * 512, 512)])
        nc.sync.dma_start(out=xq_sb, in_=xq[:, :, :, :])
        nc.sync.dma_start(out=wvp_sb, in_=wvp.rearrange("(c p) o -> p c o", p=P))
        nc.sync.dma_start(out=bptot_sb, in_=bptot[:, :])

        # --- phase 1: r (+w chain) and C --------------------------------
        with tc.tile_pool(name="ps_c", bufs=6, space="PSUM") as ps_c, \
             tc.tile_pool(name="ps_r", bufs=2, space="PSUM") as ps_r:
            for nb in range(NSLAB):
                # r = u^T.X for this slab, then w = exp(r) straight to DRAM
                psr = ps_r.tile([1, 512], F32, tag="psr")
                for ic in range(DCH):
                    nc.tensor.matmul(
                        psr,
                        u_sb[:, ic : ic + 1],
                        x_tiles[nb][:, ic, :],
                        start=(ic == 0),
                        stop=(ic == DCH - 1),
                    )
                w_sb = small.tile([1, 512], F32, tag="wsb")
                nc.scalar.activation(out=w_sb, in_=psr, func=AF.Exp)
                nc.sync.dma_start(out=w_dram[ds(nb * 512, 512)], in_=w_sb)

                # C[:, slab] = WB^T.X[:, slab]; drain quantizes (x64) to fp8
                ctiles = [ps_c.tile([P, 512], F32, tag="psc", name="psc") for _ in range(DCH)]
                for ic in range(DCH):
                    for oc in range(DCH):
                        nc.tensor.matmul(
                            ctiles[oc],
                            wb_tiles[oc][:, ic, :],
                            x_tiles[nb][:, ic, :],
                            start=(ic == 0),
                            stop=(ic == DCH - 1),
                        )
                for oc in range(DCH):
                    dst = cq_sb[:, oc // 2, oc % 2, ds(nb * 512, 512)]
                    if oc % 2 == 0:
                        nc.scalar.activation(
                            out=dst, in_=ctiles[oc], func=AF.Copy, scale=CSCALE
                        )
                    else:
                        nc.vector.tensor_scalar_mul(
                            out=dst, in0=ctiles[oc], scalar1=CSCALE
                        )

            # w column layout + fp8 copies for the den weights
            nc.sync.dma_start(
                out=wcol, in_=w_dram.rearrange("(c p) -> p c", p=P)
            )
            nc.vector.tensor_copy(out=wq8[:, :, 0], in_=wcol)
            nc.vector.tensor_copy(out=wq8[:, :, 1], in_=wcol)

        # --- phase 2a: S (DoubleRow) + VP + den, ACT exp pairs ------------
        with tc.tile_pool(name="ps_s", bufs=3, space="PSUM") as ps_s, \
             tc.tile_pool(name="ps_vp", bufs=1, space="PSUM") as ps_vp:
            for nbh in range(2):
                pair = [None, None]
                for mc in range(NCH):
                    k, par = mc // 2, mc % 2
                    if par == 0:
                        pair = [
                            ps_s.tile([P, 2, 512], F32, tag="pss", name="pssA"),
                            ps_s.tile([P, 2, 512], F32, tag="pss", name="pssB"),
                        ]
                    # S^T rows for this mc, slabs 2*nbh and 2*nbh+1
                    for jc in range(2):
                        for i in range(2):
                            nbs = 2 * nbh + i
                            nc.tensor.matmul(
                                pair[i][:, par, :],
                                xq_sb[:, jc, :, ds(mc * P, P)],
                                cq_sb[:, jc, :, ds(nbs * 512, 512)],
                                start=(jc == 0),
                                stop=(jc == 1),
                                perf_mode=DR,
                            )
                    # VP for half the mc's in each nbh phase (PE filler)
                    if mc % 2 == nbh:
                        psv = ps_vp.tile([P, 512], F32, tag="psv")
                        for ic in range(DCH):
                            nc.tensor.matmul(
                                psv,
                                x_tiles[mc // 4][:, ic, ds((mc % 4) * P, P)],
                                wvp_sb[:, ic, :],
                                start=(ic == 0),
                                stop=(ic == DCH - 1),
                            )
                        nc.vector.tensor_scalar_mul(
                            out=vpq_sb[:, mc, :],
                            in0=psv,
                            scalar1=wcol[:, mc : mc + 1],
                        )
                    if par == 1:
                        for i in range(2):
                            nbs = 2 * nbh + i
                            nc.scalar.activation(
                                out=pt_tiles[k][:, :, ds(nbs * 512, 512)],
                                in_=pair[i][:, :, :],
                                func=AF.Exp,
                                scale=1.0 / CSCALE,
                            )

        # --- phase 2b: den + recip chain, Y (DoubleRow, oc-major) ---------
        with tc.tile_pool(name="ps_den", bufs=2, space="PSUM") as ps_den, \
             tc.tile_pool(name="ps_y", bufs=6, space="PSUM") as ps_y, \
             tc.tile_pool(name="outp", bufs=4) as outp:
            for nbs in range(NSLAB):
                pden = ps_den.tile([2, 512], F32, tag="pden")
                for k in range(NCH // 2):
                    nc.tensor.matmul(
                        pden,
                        wq8[:, 2 * k : 2 * k + 2, 0:2],
                        pt_tiles[k][:, :, ds(nbs * 512, 512)],
                        start=(k == 0),
                        stop=(k == NCH // 2 - 1),
                        perf_mode=DR,
                    )
                nc.vector.tensor_copy(
                    out=den_row[0:1, ds(nbs * 512, 512)], in_=pden[0:1, :]
                )
            nc.sync.dma_start(out=den_dram[:], in_=den_row[0:1, :])
            dcol = small.tile([P, NCH], F32R, tag="dcol")
            nc.sync.dma_start(out=dcol, in_=den_dram.rearrange("(c p) -> p c", p=P))
            rcol = small.tile([P, NCH], F32, tag="rcol")
            nc.vector.reciprocal(out=rcol, in_=dcol.bitcast(F32))
            nc.sync.dma_start(
                out=recip_dram.rearrange("(c p) -> p c", p=P), in_=rcol
            )
            for nbs in range(NSLAB):
                rd = recip_dram[ds(nbs * 512, 512)]
                nc.sync.dma_start(
                    out=recip_bc[nbs],
                    in_=bass.AP(
                        tensor=rd.tensor, offset=rd.offset,
                        ap=[[0, P], rd.ap[-1]],
                    ),
                )
            for oc in range(DCH):
                ytiles = [ps_y.tile([P, 512], F32, tag="psy", name="psy") for _ in range(NSLAB)]
                for k in range(NCH // 2):
                    for nbs in range(NSLAB):
                        nc.tensor.matmul(
                            ytiles[nbs],
                            vpq_sb[:, 2 * k : 2 * k + 2, ds(oc * P, P)],
                            pt_tiles[k][:, :, ds(nbs * 512, 512)],
                            start=(k == 0),
                            stop=False,
                            perf_mode=DR,
                            skip_group_check=True,
                        )
                for nbs in range(NSLAB):
                    # += bp_tot (x) den  (rank-1), exact bias after normalize
                    nc.tensor.matmul(
                        ytiles[nbs],
                        bptot_sb[0:1, ds(oc * P, P)],
                        den_row[0:1, ds(nbs * 512, 512)],
                        start=False,
                        stop=True,
                        skip_group_check=True,
                    )
                    t = outp.tile([P, 512], F32, tag="out")
                    nc.vector.tensor_tensor(
                        out=t, in0=ytiles[nbs], in1=recip_bc[nbs],
                        op=mybir.AluOpType.mult,
                    )
                    nc.sync.dma_start(
                        out=y[ds(oc * P, P), ds(nbs * 512, 512)], in_=t
                    )

    split_sync_waits(nc)
    return nc


import numpy as np
import ml_dtypes

from concourse.bass_utils import run_bass_kernel_spmd

B = 8
E4M3 = ml_dtypes.float8_e4m3fn

_NC_CACHE = None


def _get_nc():
    global _NC_CACHE
    if _NC_CACHE is None:
        _NC_CACHE = build_nc()
    return _NC_CACHE


def _make_in_maps(inputs):
    x = np.asarray(inputs["x"], np.float32)
    W_qkv = np.asarray(inputs["W_qkv"], np.float64)
    b_qkv = np.asarray(inputs["b_qkv"], np.float64)
    W_proj = np.asarray(inputs["W_proj"], np.float64)
    b_proj = np.asarray(inputs["b_proj"], np.float64)

    s = 1.0 / np.sqrt(np.float64(D))
    wq_s = W_qkv[:, :D] * s
    bq_s = b_qkv[:D] * s
    wk = W_qkv[:, D : 2 * D]
    wv = W_qkv[:, 2 * D :]
    bv = b_qkv[2 * D :]

    shared = {
        "wb": np.ascontiguousarray(
            (wq_s @ wk.T).astype(np.float32)
            .reshape(4, 128, 4, 128).transpose(2, 1, 0, 3)
        ),
        "wvp": np.ascontiguousarray((wv @ W_proj).astype(np.float32)),
        "u": np.ascontiguousarray((wk @ bq_s).astype(np.float32)),
        "bptot": np.ascontiguousarray(
            (b_proj + bv @ W_proj).astype(np.float32).reshape(1, D)
        ),
    }
    maps = []
    for b in range(B):
        xb = np.ascontiguousarray(x[b])
        xq = np.ascontiguousarray(
            xb.astype(E4M3).reshape(2, 2, P, N).transpose(2, 0, 1, 3)
        )
        maps.append({"x": xb, "xq": xq, **shared})
    return maps


def kernel(**inputs):
    nc = _get_nc()
    in_maps = _make_in_maps(inputs)
    res = run_bass_kernel_spmd(nc, in_maps, core_ids=list(range(B)))
    return np.stack([res.results[b]["y"] for b in range(B)]).astype(np.float32)
